# revision 1
# baseline (speedup 1.0000x reference)
"""Trainium2 Bass kernel for nn_AttentionHawkes (B=32, L=2048, D=2048, 8 cores).

Sharding: batch-parallel for context/delta_t/attn (4 batches per core),
output-dim-parallel for W_in/W_out (256 rows per core). q is exchanged with
one AllToAll, combined (mix_sum|q) with one AllGather.

Per batch on its core:
  pass A: scores[l] = x[l,:]@q via DVE scalar_tensor_tensor accumulate
          (full fp32), softmax via ACT exp + gpsimd partition_all_reduce.
  pass B: mix_sum[d] = sum_l CP[l]*relu(x[l,d]) + CN[l]*relu(-x[l,d])
          with CP = attn + max(c2,0), CN = max(-c2,0) - attn,
          c2 = ae*attn*exp(-ab*dt); f32r matvecs on the PE contracting l.
Final: out[:, eslice] = tanh(combined_all @ W_out[eslice].T) via f32r matmuls.
"""
import sys, os
sys.path.insert(0, "/opt/trn_rl_repo")
import numpy as np

N_CORES = 8
B, L, D = 32, 2048, 2048
BLOC = B // N_CORES          # 4 batches per core
ESL = D // N_CORES           # 256 e-rows of W_in / W_out per core
NLT = L // 128               # 16 l-tiles per batch
NDC = D // 512               # 4 d-chunks of 512

_nc_cache = None


def _enable_ldw_opt():
    """Re-enable walrus's LDWEIGHTS dedup. Measured 2x SLOWER on HW
    (930us vs 490us) -- left here unused as a documented dead end."""
    if os.environ.get("NO_LDW_OPT", "0") == "1":
        return
    import concourse.bass_utils as bu
    if getattr(bu, "_ldw_opt_patched", False):
        return
    orig = bu.run_command

    def patched(cmd, *a, **kw):
        cmd = [c.replace("--enable-ldw-opt=false", "--enable-ldw-opt=true")
               if isinstance(c, str) else c for c in cmd]
        return orig(cmd, *a, **kw)

    bu.run_command = patched
    bu._ldw_opt_patched = True


def _build(stage=None):
    if stage is None:
        stage = int(os.environ.get('KSTAGE', '2'))
    sub = int(os.environ.get('KSUB', '9'))
    import concourse.mybir as mybir
    import concourse.tile as tile
    from concourse import bacc
    from concourse.masks import make_identity
    from concourse.bass_isa import ReduceOp

    F32 = mybir.dt.float32
    F32R = mybir.dt.float32r
    BF16 = mybir.dt.bfloat16
    ALU = mybir.AluOpType
    ACTF = mybir.ActivationFunctionType
    AX = mybir.AxisListType

    nc = bacc.Bacc()

    ctx = nc.dram_tensor("ctx", [BLOC, L, D], F32, kind="ExternalInput")
    qry = nc.dram_tensor("qry", [B, D], F32, kind="ExternalInput")
    w_in = nc.dram_tensor("w_in", [ESL, D], F32, kind="ExternalInput")
    w_out = nc.dram_tensor("w_out", [ESL, 2 * D], F32, kind="ExternalInput")
    dtt_in = nc.dram_tensor("dt", [BLOC, L], F32, kind="ExternalInput")
    aeab = nc.dram_tensor("aeab", [BLOC, 2], F32, kind="ExternalInput")

    out_sl = nc.dram_tensor("out_sl", [B, ESL], F32, kind="ExternalOutput")
    attn_out = nc.dram_tensor("attn_out", [BLOC, L], F32, kind="ExternalOutput")

    qg_in = nc.dram_tensor("qg_in", [B, ESL], F32)
    qg_out = nc.dram_tensor("qg_out", [N_CORES, BLOC, ESL], F32)
    comb_in = nc.dram_tensor("comb_in", [BLOC, 2 * D], F32)
    comb_all = nc.dram_tensor("comb_all", [B, 2 * D], F32, addr_space="Shared")
    woutT_dram = nc.dram_tensor("woutT_dram", [2 * D // 128, 128, ESL], F32)

    groups = [list(range(N_CORES))]

    with tile.TileContext(nc) as tc:
        with tc.tile_pool(name="cpool", bufs=1) as cpool:
            ident = cpool.tile([128, 128], F32)
            make_identity(nc, ident[:])
            ones_row = cpool.tile([1, 128], F32)
            nc.vector.memset(ones_row[:], 1.0)
            ones_col = cpool.tile([128, 1], F32)
            nc.vector.memset(ones_col[:], 1.0)

            # ---------- startup: weight transposes + q + AllToAll ----------
            with (
                tc.tile_pool(name="wstage", bufs=2) as wstage,
                tc.tile_pool(name="winT", bufs=NLT) as winT_pool,
                tc.tile_pool(name="qstage", bufs=1) as qstage,
                tc.tile_pool(name="wostage", bufs=2) as wostage,
                tc.tile_pool(name="wtsb", bufs=2) as wtsb_pool,
                tc.tile_pool(name="pstart", bufs=2, space="PSUM") as pstart,
            ):
                # W_in [256, 2048] -> w_inT tiles [d=128, e=256] f32 (16 tiles)
                winT = []
                for dc in range(NLT):
                    winT.append(winT_pool.tile([128, ESL], F32, tag="winT", name=f"winT{dc}"))
                wtiles = []
                for et in range(ESL // 128):
                    wt = wstage.tile([128, D], F32)
                    nc.sync.dma_start(wt[:], w_in[et * 128:(et + 1) * 128, :])
                    wtiles.append(wt)
                for et in range(ESL // 128):
                    for dc in range(NLT):
                        pt = pstart.tile([128, 128], F32, tag="pst")
                        nc.tensor.transpose(
                            pt[:], wtiles[et][:, dc * 128:(dc + 1) * 128], ident[:])
                        nc.scalar.copy(winT[dc][:, et * 128:(et + 1) * 128], pt[:])

                # queryT tiles [d=128, b=32] packed in one [128, 512] f32
                qs = qstage.tile([B, D], F32)
                nc.sync.dma_start(qs[:], qry[:])
                qT = qstage.tile([128, NLT * B], F32)
                for dc in range(NLT):
                    ptq = pstart.tile([128, 128], F32, tag="pst")
                    nc.tensor.transpose(
                        ptq[:, 0:B], qs[:, dc * 128:(dc + 1) * 128],
                        ident[0:B, 0:B])
                    nc.scalar.copy(qT[:, dc * B:(dc + 1) * B], ptq[:, 0:B])

                # q_local [32, 256] = query @ W_in[eslice].T in full fp32
                pq = pstart.tile([B, ESL], F32, tag="pq")
                for dc in range(NLT):
                    nc.tensor.matmul(
                        pq[:], qT[:, dc * B:(dc + 1) * B], winT[dc][:],
                        start=(dc == 0), stop=(dc == NLT - 1))
                q_sb = qstage.tile([B, ESL], F32)
                nc.scalar.copy(q_sb[:], pq[:])
                nc.sync.dma_start(qg_in[:], q_sb[:])
                nc.gpsimd.collective_compute(
                    "AllToAll", ALU.bypass, replica_groups=groups,
                    ins=[qg_in.ap().opt()], outs=[qg_out.ap().opt()])

                # W_out [256, 4096] -> transposed f32r tiles staged in DRAM
                wotiles = []
                for et in range(ESL // 128):
                    wo = wostage.tile([128, 2 * D], F32)
                    nc.sync.dma_start(wo[:], w_out[et * 128:(et + 1) * 128, :])
                    wotiles.append(wo)
                for ct in range(2 * D // 128):
                    wt_sb = wtsb_pool.tile([128, ESL], F32R, tag="wtsb")
                    for et in range(ESL // 128):
                        ptw = pstart.tile([128, 128], F32, tag="pst")
                        nc.tensor.transpose(
                            ptw[:], wotiles[et][:, ct * 128:(ct + 1) * 128],
                            ident[:])
                        nc.vector.tensor_copy(
                            wt_sb[:, et * 128:(et + 1) * 128], ptw[:])
                    nc.sync.dma_start(woutT_dram[ct], wt_sb[:].bitcast(F32))

            # ---------- main pools ----------
            with (
                tc.tile_pool(name="xp", bufs=NLT) as xp,
                tc.tile_pool(name="rp", bufs=2) as rp,
                tc.tile_pool(name="rn", bufs=2) as rn,
                tc.tile_pool(name="scr", bufs=1) as scr_pool,
                tc.tile_pool(name="qb", bufs=2) as qb_pool,
                tc.tile_pool(name="small", bufs=2) as small,
                tc.tile_pool(name="fin", bufs=1) as fin,
                tc.tile_pool(name="wl", bufs=2) as wl_pool,
                tc.tile_pool(name="pm", bufs=1, space="PSUM") as pm_pool,
                tc.tile_pool(name="ptr", bufs=1, space="PSUM") as ptr_pool,
                tc.tile_pool(name="pfin", bufs=1, space="PSUM") as pfin_pool,
            ):
                for b in range(BLOC if stage >= 1 else 0):
                    # q broadcast row: [128, 2048], partitions identical
                    qb = qb_pool.tile([128, D], F32, tag="qb")
                    for i in range(N_CORES):
                        nc.sync.dma_start(
                            qb[:, i * ESL:(i + 1) * ESL],
                            qg_out[i:i + 1, b, :].broadcast_to([128, ESL]))

                    # bt coefficient prep (independent of softmax)
                    dts = small.tile([NLT, 128], F32, tag="dts")
                    nc.sync.dma_start(
                        dts[:], dtt_in[b].rearrange("(t p) -> t p", p=128))
                    pdt = ptr_pool.tile([128, NLT], F32, tag="ptr")
                    nc.tensor.transpose(pdt[:], dts[:], ident[0:NLT, 0:NLT])
                    dtt = small.tile([128, NLT], F32, tag="dtt")
                    nc.scalar.copy(dtt[:], pdt[:])
                    ae_b = small.tile([128, 1], F32, tag="ae_b")
                    nc.sync.dma_start(ae_b[:],
                                      aeab[b:b + 1, 0:1].broadcast_to([128, 1]))
                    ab_b = small.tile([128, 1], F32, tag="ab_b")
                    nc.sync.dma_start(ab_b[:],
                                      aeab[b:b + 1, 1:2].broadcast_to([128, 1]))
                    negab = small.tile([128, 1], F32, tag="negab")
                    nc.vector.tensor_scalar_mul(negab[:], ab_b[:], -1.0)
                    bt = small.tile([128, NLT], F32, tag="bt")
                    nc.scalar.activation(bt[:], dtt[:], ACTF.Exp,
                                         scale=negab[:])

                    # pass A: load x tiles + scores
                    scores = small.tile([128, NLT], F32, tag="scores")
                    xts = []
                    for t in range(NLT):
                        xt = xp.tile([128, D], F32, tag="xt")
                        nc.sync.dma_start(xt[:], ctx[b, t * 128:(t + 1) * 128, :])
                        if sub >= 2:
                            scr = scr_pool.tile([128, D], BF16, tag="scr")
                            nc.vector.scalar_tensor_tensor(
                                out=scr[:], in0=xt[:], scalar=1.0, in1=qb[:],
                                op0=ALU.mult, op1=ALU.mult,
                                accum_out=scores[:, t:t + 1])
                        else:
                            nc.vector.reduce_sum(scores[:, t:t + 1], xt[:],
                                                 axis=AX.X)
                        xts.append(xt)

                    # softmax over all 2048 scores (PE partition reduces)
                    if sub < 3:
                        continue
                    m1 = small.tile([128, 1], F32, tag="m1")
                    nc.vector.reduce_max(m1[:], scores[:], axis=AX.X)
                    ptm = ptr_pool.tile([1, 128], F32, tag="ptr")
                    nc.tensor.transpose(ptm[:], m1[:], ident[:])
                    mg = small.tile([1, 1], F32, tag="mg")
                    nc.vector.reduce_max(mg[:], ptm[:], axis=AX.X)
                    nc.vector.tensor_scalar_mul(mg[:], mg[:], -1.0)
                    pnb = ptr_pool.tile([128, 1], F32, tag="ptr2")
                    nc.tensor.matmul(pnb[:], ones_row[:], mg[:],
                                     start=True, stop=True)
                    negm = small.tile([128, 1], F32, tag="negm")
                    nc.scalar.copy(negm[:], pnb[:])
                    E = small.tile([128, NLT], F32, tag="E")
                    s1 = small.tile([128, 1], F32, tag="s1")
                    nc.scalar.activation(E[:], scores[:], ACTF.Exp,
                                         bias=negm[:], accum_out=s1[:])
                    pz = ptr_pool.tile([1, 1], F32, tag="ptr")
                    nc.tensor.matmul(pz[:], s1[:, 0:1], ones_col[:, 0:1],
                                     start=True, stop=True)
                    rzg = small.tile([1, 1], F32, tag="rzg")
                    nc.vector.reciprocal(rzg[:], pz[:])
                    prz = ptr_pool.tile([128, 1], F32, tag="ptr2")
                    nc.tensor.matmul(prz[:], ones_row[:], rzg[:],
                                     start=True, stop=True)
                    rz = small.tile([128, 1], F32, tag="rz")
                    nc.scalar.copy(rz[:], prz[:])
                    attn = small.tile([128, NLT], F32, tag="attn")
                    nc.vector.tensor_scalar(out=attn[:], in0=E[:],
                                            scalar1=rz[:], scalar2=None,
                                            op0=ALU.mult)

                    # coefficients CP, CN
                    if sub < 4:
                        continue
                    c2 = small.tile([128, NLT], F32, tag="c2")
                    nc.vector.tensor_tensor(out=c2[:], in0=attn[:], in1=bt[:],
                                            op=ALU.mult)
                    nc.vector.tensor_scalar(out=c2[:], in0=c2[:],
                                            scalar1=ae_b[:], scalar2=None,
                                            op0=ALU.mult)
                    cp = small.tile([128, NLT], F32, tag="cp")
                    nc.vector.tensor_scalar(out=cp[:], in0=c2[:], scalar1=0.0,
                                            scalar2=None, op0=ALU.max)
                    cp_r = small.tile([128, NLT], F32R, tag="cp_r")
                    nc.vector.tensor_tensor(out=cp_r[:], in0=cp[:],
                                            in1=attn[:], op=ALU.add)
                    cn = small.tile([128, NLT], F32, tag="cn")
                    nc.vector.tensor_scalar(out=cn[:], in0=c2[:], scalar1=-1.0,
                                            scalar2=0.0, op0=ALU.mult,
                                            op1=ALU.max)
                    cn_r = small.tile([128, NLT], F32R, tag="cn_r")
                    nc.vector.tensor_tensor(out=cn_r[:], in0=cn[:],
                                            in1=attn[:], op=ALU.subtract)

                    # pass B: accumulate CP*relu(x) + CN*relu(-x) over l
                    if sub < 5:
                        continue
                    pms = [pm_pool.tile([2, 512], F32, tag=f"pm{dc}", name=f"pm{dc}")
                           for dc in range(NDC)]
                    for t in range(NLT):
                        rpl = rp.tile([128, D], F32R, tag="rpl")
                        nc.scalar.activation(rpl[:], xts[t][:], ACTF.Relu)
                        rnl = rn.tile([128, D], F32R, tag="rnl")
                        nc.vector.tensor_scalar(out=rnl[:], in0=xts[t][:],
                                                scalar1=-1.0, scalar2=0.0,
                                                op0=ALU.mult, op1=ALU.max)
                        for dc in range(NDC):
                            nc.tensor.matmul(
                                pms[dc][:],
                                cp_r[:, t:t + 1].broadcast_to([128, 2]),
                                rpl[:, dc * 512:(dc + 1) * 512],
                                start=(t == 0), stop=False)
                        for dc in range(NDC):
                            nc.tensor.matmul(
                                pms[dc][:],
                                cn_r[:, t:t + 1].broadcast_to([128, 2]),
                                rnl[:, dc * 512:(dc + 1) * 512],
                                start=False, stop=(t == NLT - 1))

                    # combined row -> DRAM
                    if sub < 6:
                        continue
                    ms = fin.tile([1, D], F32, tag="ms")
                    for dc in range(NDC):
                        nc.scalar.copy(ms[:, dc * 512:(dc + 1) * 512],
                                       pms[dc][0:1, :])
                    nc.sync.dma_start(comb_in[b:b + 1, 0:D], ms[0:1, :])
                    nc.sync.dma_start(comb_in[b:b + 1, D:2 * D], qb[0:1, :])

                    # attn output (transpose to l-major)
                    if sub < 7:
                        continue
                    pat = ptr_pool.tile([NLT, 128], F32, tag="ptr")
                    nc.tensor.transpose(pat[:], attn[:], ident[:])
                    at_sb = small.tile([NLT, 128], F32, tag="at_sb")
                    nc.scalar.copy(at_sb[:], pat[:])
                    nc.sync.dma_start(
                        attn_out[b].rearrange("(t p) -> t p", p=128), at_sb[:])

                if stage >= 2:
                    # ---------- final: W_out matvec over gathered combined ----
                    nc.gpsimd.collective_compute(
                        "AllGather", ALU.bypass, replica_groups=groups,
                        ins=[comb_in.ap().opt()], outs=[comb_all.ap().opt()])
                    combT = []
                    for ct in range(2 * D // 128):
                        cst = wl_pool.tile([B, 128], F32, tag="cst")
                        nc.sync.dma_start(cst[:],
                                          comb_all[:, ct * 128:(ct + 1) * 128])
                        ptc = ptr_pool.tile([128, B], F32, tag="ptr")
                        nc.tensor.transpose(ptc[:], cst[:], ident[0:B, 0:B])
                        cT = fin.tile([128, B], F32R, tag=f"cT{ct}",
                                      name=f"cT{ct}")
                        nc.vector.tensor_copy(cT[:], ptc[:])
                        combT.append(cT)
                    n_ct = 2 * D // 128
                    pos = [pfin_pool.tile([128, B], F32, tag=f"po{ec}",
                                          name=f"po{ec}")
                           for ec in range(ESL // 128)]
                    for ct in range(n_ct):
                        wlf = wl_pool.tile([128, ESL], F32, tag="wlf",
                                           name=f"wlf{ct}")
                        nc.sync.dma_start(wlf[:], woutT_dram[ct])
                        wlt = wl_pool.tile([128, ESL], F32R, tag="wlt",
                                           name=f"wlt{ct}")
                        nc.vector.tensor_copy(wlt[:], wlf[:])
                        for ec in range(ESL // 128):
                            nc.tensor.matmul(
                                pos[ec][:], wlt[:, ec * 128:(ec + 1) * 128],
                                combT[ct][:],
                                start=(ct == 0), stop=(ct == n_ct - 1))
                    for ec in range(ESL // 128):
                        oo = small.tile([128, B], F32, tag="oo")
                        nc.scalar.activation(oo[:], pos[ec][:], ACTF.Tanh)
                        pto = ptr_pool.tile([B, 128], F32, tag="ptr")
                        nc.tensor.transpose(pto[:], oo[:], ident[:])
                        os_sb = small.tile([B, 128], F32, tag="os_sb")
                        nc.scalar.copy(os_sb[:], pto[:])
                        nc.sync.dma_start(out_sl[:, ec * 128:(ec + 1) * 128],
                                          os_sb[:])
    nc.finalize()
    return nc


def _get_nc():
    global _nc_cache
    if _nc_cache is None:
        _nc_cache = _build()
    return _nc_cache


def _make_in_maps(inputs):
    query = np.ascontiguousarray(
        np.asarray(inputs["query"], np.float32).reshape(B, D))
    context = np.ascontiguousarray(np.asarray(inputs["context"], np.float32))
    delta_t = np.ascontiguousarray(np.asarray(inputs["delta_t"], np.float32))
    W_in = np.ascontiguousarray(np.asarray(inputs["W_in"], np.float32))
    W_out = np.ascontiguousarray(np.asarray(inputs["W_out"], np.float32))
    aeab_full = np.concatenate(
        [np.asarray(inputs["ae"], np.float32).reshape(B, 1),
         np.asarray(inputs["ab"], np.float32).reshape(B, 1)], axis=1)
    in_maps = []
    for c in range(N_CORES):
        in_maps.append({
            "ctx": context[c * BLOC:(c + 1) * BLOC],
            "qry": query,
            "w_in": np.ascontiguousarray(W_in[c * ESL:(c + 1) * ESL]),
            "w_out": np.ascontiguousarray(W_out[c * ESL:(c + 1) * ESL]),
            "dt": np.ascontiguousarray(delta_t[c * BLOC:(c + 1) * BLOC]),
            "aeab": np.ascontiguousarray(aeab_full[c * BLOC:(c + 1) * BLOC]),
        })
    return in_maps


def kernel(query, context, delta_t, W_in, W_out, ae, ab):
    from concourse.bass_utils import run_bass_kernel_spmd

    nc = _get_nc()
    in_maps = _make_in_maps(dict(query=query, context=context,
                                 delta_t=delta_t, W_in=W_in, W_out=W_out,
                                 ae=ae, ab=ab))
    res = run_bass_kernel_spmd(nc, in_maps, list(range(N_CORES))).results

    out = np.concatenate([res[c]["out_sl"] for c in range(N_CORES)], axis=1)
    attn = np.concatenate([res[c]["attn_out"] for c in range(N_CORES)], axis=0)
    return out.reshape(B, 1, D), attn.reshape(B, 1, L)

